# revision 69
# baseline (speedup 1.0000x reference)
"""BERT self-attention (B=8, S=1024, D=1024, H=16, DH=64) on 8 Trainium2 cores.

Strategy: pure data-parallel over batch - each of the 8 cores runs the full
self-attention for one batch element. No collectives.

Single software-pipelined stream so the ACT engine's exp work (~134us/core,
1 elem/lane/cycle, irreducible) hides under the PE's matmul work instead of
serializing after it (sim: PE and ACT both 99-100% busy in steady state):

  X^T (PE transposes of fp32 X, psum->sbuf copies convert to bf16)
  jt=0: Q^T/K^T proj -> scores+exp heads 0,1
  jt=1: proj -> V tiles interleaved with scores+exp heads 2,3
  jt=2..7: proj -> scores+exp heads 2jt,2jt+1, with the qt-blocks of
           ctx[h-3] interleaved between scores kt-blocks; the lag catches
           down to 1 at the tail so only ctx[15] runs after exp[15]

Key scheduling decisions (each HW-measured):
  - Wq/Wk arrive as per-jt column blocks (one 3D-AP DMA each) so proj jt=0
    needs just 1MB of W beyond X: the startup is DMA-roofline-bound (16MB
    of fp32 inputs) and exp starts ~20us earlier than whole-matrix
    delivery. DMA instruction count is kept low (flat HWDGE cost/DMA):
    one DMA per W column-block, one output DMA per head.
  - Projections accumulate in their own 1-bank [128,512] PSUM tag. Sharing
    the scores ring made proj wait on exp completions (engine FIFOs +
    2-deep psum ring), starving ACT during projections: -19us HW.
  - PSUM (8 banks): scores [128,1024]x2, proj [128,512]x2, ctx [128,65]x2.
  - V bias rides the PSUM->SBUF copy (scalar_tensor_tensor add of a
    precomputed ones x bv broadcast): -27us HW vs rank-1 bias matmuls that
    broke the PSUM accumulation groups.

Datapath is bf16 end-to-end on the PE (weights, X^T, Q^T, K^T, V, probs):
1 cycle/row matmuls everywhere plus fast weight loads (FWL needs non-fp32
dtype). PSUM accumulation stays fp32; rel err vs fp32 reference ~3.1e-3.
W/X arrive fp32 via DMA and are converted on the DVE. Q/K biases are folded
into the PSUM->SBUF copy as per-partition tensor_scalar adds (j sits on
partitions in Q^T/K^T). The attention mask (indexed by k) is a
per-partition bias folded with the 1/sqrt(DH) scale into the Exp activation
on transposed scores S^T[k,q]. The V tiles carry a ones column per head so
the context matmul emits the softmax denominator for free; DVE reciprocal +
tensor_scalar multiply normalize, writing a per-head bounce tile that goes
out in a single DMA.

Built on bacc.Bacc: its compile() legalizes sync waits (1 wait/instruction
hardware limit) via move_matmul_waits_to_ldweights + generate_event_semaphores.
"""

import numpy as np

import concourse.bass as bass
import concourse.bacc as bacc
import concourse.mybir as mybir
import concourse.tile as tile
from concourse.bass_utils import run_bass_kernel_spmd
from concourse.masks import make_identity

F32 = mybir.dt.float32
F32R = mybir.dt.float32r
BF16 = mybir.dt.bfloat16

B, S, D, H = 8, 1024, 1024, 16
DH = D // H  # 64
P = 128
NT = S // P  # 8 tiles along any 1024 dim
SC = S // 512  # 2 chunks of 512
SCALE = 1.0 / float(np.sqrt(DH))
N_CORES = 8
VW = DH + 1  # 65: V block width per head (64 cols + ones col)
CTX_LAG = 3  # ctx[h-CTX_LAG] is emitted after scores/exp[h]


def emit_body(nc, dram, pools):
    (x_d, m_d, wq_d, bq_d, wk_d, bk_d, wv_d, bv_d, o_d) = dram
    (cst, xT_pool, w_pool, qk_pool, v_pool, x_stage, w_stage, wv_stage,
     p_pool, small_pool, bounce_pool, ps_sps, ps_proj, ps_ctx, ident) = pools

    # ---- input DMAs (front-loaded so the DGE rings start immediately;
    # staging pool depths pace them) ----
    x_t = []

    def dma_x(st):
        t = x_stage.tile([P, D], F32, name="x_tile", tag="xs")
        nc.sync.dma_start(out=t, in_=x_d.ap()[st * P : (st + 1) * P, :])
        x_t.append(t)

    nonlocal_store = {}
    b_cols = {}

    def emit_consts():
        # tiny gathers; after X so x0 isn't delayed, before the W streams
        # so the bias/mask consumers aren't starved
        m = cst.tile([P, NT], F32, name="mask_cols", tag="mask_cols")
        nc.sync.dma_start(out=m, in_=m_d.ap().rearrange("(g p) -> p g", p=P))
        nonlocal_store["mask_cols"] = m
        for nm, hd in (("bq", bq_d), ("bk", bk_d)):
            t = cst.tile([P, NT], F32, name=f"bcol_{nm}", tag=f"bcol_{nm}")
            nc.sync.dma_start(out=t, in_=hd.ap().rearrange("(g p) -> p g", p=P))
            b_cols[nm] = t
        bv_f32 = cst.tile([1, D], F32, name="bv_f32", tag="bv_f32")
        nc.sync.dma_start(out=bv_f32, in_=bv_d.ap().unsqueeze(0))
        nc.vector.tensor_copy(bv_row, bv_f32)

    bv_row = cst.tile([1, D], BF16, name="bv_row", tag="bv_row")
    ones_f32 = cst.tile([1, P], F32, name="ones_f32", tag="ones_f32")
    nc.vector.memset(ones_f32, 1.0)
    ones_row = cst.tile([1, P], BF16, name="ones_row", tag="ones_row")
    nc.vector.tensor_copy(ones_row, ones_f32)

    # Wq/Wk arrive as per-jt column blocks (one [128, it*128] DMA each) so
    # projection jt needs only 1MB of W beyond X - scores/exp start ~20us
    # earlier than with whole-matrix delivery. Wv arrives as full rows (V
    # streams them as rhs).
    wjt_f32 = {}  # (nm, jt) -> [128, 1024] f32 tile, it-major columns

    def dma_wjt(jt):
        for nm, w_d in (("wq", wq_d), ("wk", wk_d)):
            t = w_stage.tile([P, NT * P], F32, name=f"{nm}jt", tag="wjt")
            nc.sync.dma_start(
                out=t.rearrange("p (i j) -> p i j", j=P),
                in_=w_d.ap()
                .rearrange("(i p) d -> p i d", p=P)[:, :, jt * P : (jt + 1) * P],
            )
            wjt_f32[(nm, jt)] = t

    for st in range(NT):
        dma_x(st)
    emit_consts()
    dma_wjt(0)
    dma_wjt(1)
    wv_f32 = []
    for it in range(NT):
        t = wv_stage.tile([P, D], F32, name="wvf", tag="ws")
        nc.sync.dma_start(out=t, in_=wv_d.ap()[it * P : (it + 1) * P, :])
        wv_f32.append(t)
    for jt in range(2, NT):
        dma_wjt(jt)

    # ---- phase 1: X^T via PE transposes (fp32 in, bf16 out via DVE) ----
    xT = []
    for it in range(NT):
        xT.append(xT_pool.tile([P, S], BF16, name=f"xT{it}", tag=f"xT{it}"))

    def emit_xt(st_lo, st_hi):
        for st in range(st_lo, st_hi):
            for it in range(NT):
                pt = ps_proj.tile([P, P], F32, name="pt", tag="proj")
                nc.tensor.transpose(pt, x_t[st][:, it * P : (it + 1) * P], ident)
                nc.vector.tensor_copy(xT[it][:, st * P : (st + 1) * P], pt)

    # just-in-time weight conversions (DVE)
    w_bf = {}  # (nm, jt) -> [128, 1024] bf16 tile, it-major columns

    def cvt_wjt(jt):
        for nm in ("wq", "wk"):
            t = w_pool.tile([P, NT * P], BF16, name=f"{nm}b", tag=f"{nm}b{jt}")
            nc.vector.tensor_copy(t, wjt_f32[(nm, jt)])
            w_bf[(nm, jt)] = t

    wv_tiles = []

    def cvt_wv():
        for it in range(NT):
            t = w_pool.tile([P, D], BF16, name="wvb", tag=f"wvb{it}")
            nc.vector.tensor_copy(t, wv_f32[it])
            wv_tiles.append(t)

    # ---- V tiles (bf16, head-interleaved 65-col blocks w/ ones col) ----
    v_sb = []
    for st in range(NT):
        v = v_pool.tile([P, H * VW], BF16, name=f"v{st}", tag=f"v{st}")
        nc.gpsimd.memset(v, 1.0)  # ones columns survive at h*65+64
        v_sb.append(v)

    # bv broadcast across partitions (ones x bv rank-1, computed once) so
    # the per-st bias add rides the PSUM->SBUF copy instead of 16 matmuls.
    bvb = cst.tile([P, D], BF16, name="bvb", tag="bvb")

    def emit_bvb():
        mm = ps_sps.tile([P, S], F32, name="mmb", tag="sps")
        for jc in range(SC):
            nc.tensor.matmul(
                mm[:, jc * 512 : (jc + 1) * 512],
                lhsT=ones_row[0:1, 0:P],
                rhs=bv_row[0:1, jc * 512 : (jc + 1) * 512],
                start=True,
                stop=True,
            )
        nc.vector.tensor_copy(bvb, mm)

    def emit_v_tile(st):
        mm = ps_sps.tile([P, S], F32, name="mmv", tag="sps")
        for it in range(NT):
            lhsT = xT[it][:, st * P : (st + 1) * P]
            for jc in range(SC):
                nc.tensor.matmul(
                    mm[:, jc * 512 : (jc + 1) * 512],
                    lhsT=lhsT,
                    rhs=wv_tiles[it][:, jc * 512 : (jc + 1) * 512],
                    start=(it == 0),
                    stop=(it == NT - 1),
                )
        dst = v_sb[st].rearrange("p (g c) -> p g c", c=VW)[:, :, 0:DH]
        nc.vector.scalar_tensor_tensor(
            out=dst,
            in0=mm.rearrange("p (g c) -> p g c", c=DH),
            scalar=0.0,
            in1=bvb.rearrange("p (g c) -> p g c", c=DH),
            op0=mybir.AluOpType.add,
            op1=mybir.AluOpType.add,
        )

    # ---- projections for one jt (Q^T and K^T rows jt*128..) ----
    qk_tiles = {}  # jt -> (qT, kT)

    def emit_proj(jt, scs=(0, 1)):
        if jt not in qk_tiles:
            qk_tiles[jt] = tuple(
                qk_pool.tile([P, S], BF16, name=f"{nm}T", tag="qk")
                for nm in ("bq", "bk")
            )
        for dst, nm, wkey in zip(qk_tiles[jt], ("bq", "bk"), ("wq", "wk")):
            # per-half psum tiles (own 1-bank tag: proj never waits on the
            # scores ring, so ACT keeps draining exps during projections)
            for sc in scs:
                mm = ps_proj.tile([P, 512], F32, name="mmp", tag="proj")
                for it in range(NT):
                    nc.tensor.matmul(
                        mm,
                        lhsT=w_bf[(wkey, jt)][:, it * P : (it + 1) * P],
                        rhs=xT[it][:, sc * 512 : (sc + 1) * 512],
                        start=(it == 0),
                        stop=(it == NT - 1),
                    )
                nc.vector.tensor_scalar_add(
                    dst[:, sc * 512 : (sc + 1) * 512],
                    mm,
                    b_cols[nm][:, jt : jt + 1],
                )

    # ---- scores + exp for one head (S^T[k,q] by kt tile) ----
    pT_store = {}  # h -> list of 8 pT tiles

    def emit_ctx_qt(h, pT, bounce, qt):
        cps = ps_ctx.tile([P, VW], F32, name="cps", tag="ctx")
        for kt in range(NT):
            nc.tensor.matmul(
                cps,
                lhsT=pT[kt][:, qt * P : (qt + 1) * P],
                rhs=v_sb[kt][:, h * VW : (h + 1) * VW],
                start=(kt == 0),
                stop=(kt == NT - 1),
            )
        r = small_pool.tile([P, 1], F32, name="recip", tag="recip")
        nc.vector.reciprocal(r, cps[:, DH : DH + 1])
        nc.vector.tensor_scalar_mul(
            bounce[:, qt * DH : (qt + 1) * DH], cps[:, 0:DH], r
        )

    def emit_ctx_out_dma(h, bounce):
        # one DMA per head: [p, qt, j] -> out[(qt p), h*64+j]
        nc.sync.dma_start(
            out=o_d.ap()
            .rearrange("(q p) d -> p q d", p=P)[:, :, h * DH : (h + 1) * DH],
            in_=bounce.rearrange("p (q j) -> p q j", j=DH),
        )

    def emit_scores_exp(h, ctx_heads=()):
        # scores+exp for head h, with the qt-blocks of lagged ctx heads
        # interleaved between kt-blocks: PE chews ctx matmuls while ACT
        # drains the scores psum ring instead of head-of-line stalling.
        jt, ro = h // 2, (h % 2) * DH
        qT_t, kT_t = qk_tiles[jt]
        jobs = [(c, pT_store.pop(c),
                 bounce_pool.tile([P, NT * DH], F32, name="bounce", tag="bounce"))
                for c in ctx_heads]
        pT = []
        for kt in range(NT):
            sps = ps_sps.tile([P, S], F32, name="sps", tag="sps")
            lhsT = kT_t[ro : ro + DH, kt * P : (kt + 1) * P]
            for qc in range(SC):
                nc.tensor.matmul(
                    sps[:, qc * 512 : (qc + 1) * 512],
                    lhsT=lhsT,
                    rhs=qT_t[ro : ro + DH, qc * 512 : (qc + 1) * 512],
                    start=True,
                    stop=True,
                )
            pt = p_pool.tile([P, S], BF16, name="pT", tag="pT")
            nc.scalar.activation(
                pt,
                sps,
                mybir.ActivationFunctionType.Exp,
                bias=nonlocal_store["mask_cols"][:, kt : kt + 1],
                scale=SCALE,
            )
            pT.append(pt)
            for c, cpT, bounce in jobs:
                emit_ctx_qt(c, cpT, bounce, kt)
        pT_store[h] = pT
        for c, cpT, bounce in jobs:
            emit_ctx_out_dma(c, bounce)

    # ---- standalone ctx (tail) ----
    def emit_ctx(h):
        pT = pT_store.pop(h)
        bounce = bounce_pool.tile([P, NT * DH], F32, name="bounce", tag="bounce")
        for qt in range(NT):
            emit_ctx_qt(h, pT, bounce, qt)
        emit_ctx_out_dma(h, bounce)

    # ---- pipelined schedule ----
    # ctx[h] lags scores/exp[h] by CTX_LAG heads mid-stream (pT buffering),
    # catching down to lag 1 at the tail so only ctx[15] runs after exp[15].
    next_ctx = [0]

    def emit_head(h):
        lag = CTX_LAG if h < H - 4 else (2 if h < H - 2 else 1)
        ctx_heads = []
        while next_ctx[0] <= h - lag:
            ctx_heads.append(next_ctx[0])
            next_ctx[0] += 1
        emit_scores_exp(h, ctx_heads)

    emit_xt(0, NT)
    cvt_wjt(0)
    emit_proj(0)
    cvt_wjt(1)
    emit_head(0)
    cvt_wjt(2)
    emit_head(1)
    cvt_wv()
    emit_bvb()
    emit_proj(1)
    # V interleaved with heads 2,3: PE chews V while ACT drains the
    # exp backlog of heads 0,1; all of V lands before ctx[0] (h=3).
    for st in range(0, 4):
        emit_v_tile(st)
    emit_head(2)
    for st in range(4, NT):
        emit_v_tile(st)
    emit_head(3)
    for jt in range(2, NT):
        emit_proj(jt)
        if jt + 1 < NT:
            cvt_wjt(jt + 1)
        emit_head(2 * jt)
        emit_head(2 * jt + 1)
    while next_ctx[0] < H:
        emit_ctx(next_ctx[0])
        next_ctx[0] += 1


def build_program(n_reps: int = 1, n_loop: int = 0) -> bass.Bass:
    nc = bacc.Bacc(trn_type="TRN2", target_bir_lowering=False, debug=False)

    x_d = nc.declare_dram_parameter("hidden_states", [S, D], F32, isOutput=False)
    m_d = nc.declare_dram_parameter("attention_mask", [S], F32, isOutput=False)
    wq_d = nc.declare_dram_parameter("Wq", [D, D], F32, isOutput=False)
    bq_d = nc.declare_dram_parameter("bq", [D], F32, isOutput=False)
    wk_d = nc.declare_dram_parameter("Wk", [D, D], F32, isOutput=False)
    bk_d = nc.declare_dram_parameter("bk", [D], F32, isOutput=False)
    wv_d = nc.declare_dram_parameter("Wv", [D, D], F32, isOutput=False)
    bv_d = nc.declare_dram_parameter("bv", [D], F32, isOutput=False)
    o_d = nc.declare_dram_parameter("out", [S, D], F32, isOutput=True)
    dram = (x_d, m_d, wq_d, bq_d, wk_d, bk_d, wv_d, bv_d, o_d)

    with tile.TileContext(nc) as tc:
        with (
            tc.tile_pool(name="consts", bufs=1) as cst,
            tc.tile_pool(name="xT", bufs=1) as xT_pool,
            tc.tile_pool(name="wpool", bufs=1) as w_pool,
            tc.tile_pool(name="qk", bufs=6) as qk_pool,
            tc.tile_pool(name="vsb", bufs=1) as v_pool,
            tc.tile_pool(name="xstage", bufs=2) as x_stage,
            # [128,1024]f32 rings: 1.5 jt's worth of Wq+Wk column-block
            # prefetch; Wv full rows get their own ring.
            tc.tile_pool(name="wstage", bufs=3) as w_stage,
            tc.tile_pool(name="wvstage", bufs=3) as wv_stage,
            tc.tile_pool(name="pT", bufs=8 * (CTX_LAG + 1)) as p_pool,
            tc.tile_pool(name="small", bufs=16) as small_pool,
            tc.tile_pool(name="bounce", bufs=3) as bounce_pool,
            # PSUM (8 banks): scores [128,1024] x2 = 4, proj [128,512] x2
            # = 2, ctx [128,65] x2 = 2.
            tc.tile_pool(name="pssps", bufs=2, space="PSUM") as ps_sps,
            tc.tile_pool(name="psproj", bufs=2, space="PSUM") as ps_proj,
            tc.tile_pool(name="psctx", bufs=2, space="PSUM") as ps_ctx,
        ):
            ident = cst.tile([P, P], F32, name="ident", tag="ident")
            make_identity(nc, ident)
            pools = (cst, xT_pool, w_pool, qk_pool, v_pool, x_stage, w_stage,
                     wv_stage, p_pool, small_pool, bounce_pool, ps_sps,
                     ps_proj, ps_ctx, ident)
            if n_loop:
                with tc.For_i(0, n_loop, 1):
                    emit_body(nc, dram, pools)
            else:
                for _ in range(n_reps):
                    emit_body(nc, dram, pools)
    nc.compile()
    return nc


_NC_CACHE = None


def _get_nc():
    global _NC_CACHE
    if _NC_CACHE is None:
        _NC_CACHE = build_program()
    return _NC_CACHE


def make_in_maps(hidden_states, attention_mask, Wq, bq, Wk, bk, Wv, bv):
    hs = np.ascontiguousarray(np.asarray(hidden_states, dtype=np.float32))
    am = np.ascontiguousarray(
        np.asarray(attention_mask, dtype=np.float32).reshape(B, S)
    )
    shared = {
        "Wq": np.ascontiguousarray(np.asarray(Wq, dtype=np.float32)),
        "bq": np.ascontiguousarray(np.asarray(bq, dtype=np.float32)),
        "Wk": np.ascontiguousarray(np.asarray(Wk, dtype=np.float32)),
        "bk": np.ascontiguousarray(np.asarray(bk, dtype=np.float32)),
        "Wv": np.ascontiguousarray(np.asarray(Wv, dtype=np.float32)),
        "bv": np.ascontiguousarray(np.asarray(bv, dtype=np.float32)),
    }
    return [
        {"hidden_states": hs[b], "attention_mask": am[b], **shared}
        for b in range(B)
    ]


def kernel(hidden_states, attention_mask, Wq, bq, Wk, bk, Wv, bv):
    nc = _get_nc()
    in_maps = make_in_maps(hidden_states, attention_mask, Wq, bq, Wk, bk, Wv, bv)
    res = run_bass_kernel_spmd(nc, in_maps, list(range(N_CORES))).results
    out = np.stack([np.asarray(res[b]["out"], dtype=np.float32) for b in range(B)])
    return out


# revision 79
# speedup vs baseline: 1.0049x; 1.0049x over previous
"""BERT self-attention (B=8, S=1024, D=1024, H=16, DH=64) on 8 Trainium2 cores.

Strategy: pure data-parallel over batch - each of the 8 cores runs the full
self-attention for one batch element. No collectives.

Single software-pipelined stream so the ACT engine's exp work (~134us/core,
1 elem/lane/cycle, irreducible) hides under the PE's matmul work instead of
serializing after it (sim: PE and ACT both 99-100% busy in steady state):

  X^T (PE transposes of fp32 X, psum->sbuf copies convert to bf16)
  jt=0: Q^T/K^T proj -> scores+exp heads 0,1
  jt=1: proj -> V tiles interleaved with scores+exp heads 2,3
  jt=2..7: proj -> scores+exp heads 2jt,2jt+1, with the qt-blocks of
           ctx[h-3] interleaved between scores kt-blocks; the lag catches
           down to 1 at the tail so only ctx[15] runs after exp[15]

Key scheduling decisions (each HW-measured):
  - Wq/Wk arrive as per-jt column blocks (one 3D-AP DMA each) so proj jt=0
    needs just 1MB of W beyond X: the startup is DMA-roofline-bound (16MB
    of fp32 inputs) and exp starts ~20us earlier than whole-matrix
    delivery. DMA instruction count is kept low (flat HWDGE cost/DMA):
    one DMA per W column-block, one output DMA per head.
  - Projections accumulate in their own 1-bank [128,512] PSUM tag. Sharing
    the scores ring made proj wait on exp completions (engine FIFOs +
    2-deep psum ring), starving ACT during projections: -19us HW.
  - PSUM (8 banks): scores [128,1024]x2, proj [128,512]x2, ctx [128,65]x2.
  - V bias rides the PSUM->SBUF copy (scalar_tensor_tensor add of a
    precomputed ones x bv broadcast): -27us HW vs rank-1 bias matmuls that
    broke the PSUM accumulation groups.

Datapath is bf16 end-to-end on the PE (weights, X^T, Q^T, K^T, V, probs):
1 cycle/row matmuls everywhere plus fast weight loads (FWL needs non-fp32
dtype). PSUM accumulation stays fp32; rel err vs fp32 reference ~3.1e-3.
W/X arrive fp32 via DMA and are converted on the DVE. Q/K biases are folded
into the PSUM->SBUF copy as per-partition tensor_scalar adds (j sits on
partitions in Q^T/K^T). The attention mask (indexed by k) is a
per-partition bias folded with the 1/sqrt(DH) scale into the Exp activation
on transposed scores S^T[k,q]. The V tiles carry a ones column per head so
the context matmul emits the softmax denominator for free; DVE reciprocal +
tensor_scalar multiply normalize, writing a per-head bounce tile that goes
out in a single DMA.

Built on bacc.Bacc: its compile() legalizes sync waits (1 wait/instruction
hardware limit) via move_matmul_waits_to_ldweights + generate_event_semaphores.
"""

import numpy as np

import concourse.bass as bass
import concourse.bacc as bacc
import concourse.mybir as mybir
import concourse.tile as tile
from concourse.bass_utils import run_bass_kernel_spmd
from concourse.masks import make_identity

F32 = mybir.dt.float32
F32R = mybir.dt.float32r
BF16 = mybir.dt.bfloat16

B, S, D, H = 8, 1024, 1024, 16
DH = D // H  # 64
P = 128
NT = S // P  # 8 tiles along any 1024 dim
SC = S // 512  # 2 chunks of 512
SCALE = 1.0 / float(np.sqrt(DH))
N_CORES = 8
VW = DH + 1  # 65: V block width per head (64 cols + ones col)
CTX_LAG = 3  # ctx[h-CTX_LAG] is emitted after scores/exp[h]


def emit_body(nc, dram, pools):
    (x_d, m_d, wq_d, bq_d, wk_d, bk_d, wv_d, bv_d, o_d) = dram
    (cst, xT_pool, w_pool, qk_pool, v_pool, x_stage, w_stage, wv_stage,
     p_pool, small_pool, bounce_pool, ps_sps, ps_proj, ps_ctx, ident) = pools

    # ---- input DMAs (front-loaded so the DGE rings start immediately;
    # staging pool depths pace them) ----
    x_t = []

    def dma_x(st):
        t = x_stage.tile([P, D], F32, name="x_tile", tag="xs")
        nc.sync.dma_start(out=t, in_=x_d.ap()[st * P : (st + 1) * P, :])
        x_t.append(t)

    nonlocal_store = {}
    b_cols = {}

    def emit_consts():
        # tiny gathers; after X so x0 isn't delayed, before the W streams
        # so the bias/mask consumers aren't starved
        m = cst.tile([P, NT], F32, name="mask_cols", tag="mask_cols")
        nc.sync.dma_start(out=m, in_=m_d.ap().rearrange("(g p) -> p g", p=P))
        nonlocal_store["mask_cols"] = m
        for nm, hd in (("bq", bq_d), ("bk", bk_d)):
            t = cst.tile([P, NT], F32, name=f"bcol_{nm}", tag=f"bcol_{nm}")
            nc.sync.dma_start(out=t, in_=hd.ap().rearrange("(g p) -> p g", p=P))
            b_cols[nm] = t
        bv_f32 = cst.tile([1, D], F32, name="bv_f32", tag="bv_f32")
        nc.sync.dma_start(out=bv_f32, in_=bv_d.ap().unsqueeze(0))
        nc.vector.tensor_copy(bv_row, bv_f32)

    bv_row = cst.tile([1, D], BF16, name="bv_row", tag="bv_row")
    ones_f32 = cst.tile([1, P], F32, name="ones_f32", tag="ones_f32")
    nc.vector.memset(ones_f32, 1.0)
    ones_row = cst.tile([1, P], BF16, name="ones_row", tag="ones_row")
    nc.vector.tensor_copy(ones_row, ones_f32)

    # Wq/Wk arrive as per-jt column blocks (one [128, it*128] DMA each) so
    # projection jt needs only 1MB of W beyond X - scores/exp start ~20us
    # earlier than with whole-matrix delivery. Wv arrives as full rows (V
    # streams them as rhs).
    wjt_f32 = {}  # (nm, jt) -> [128, 1024] f32 tile, it-major columns

    def dma_wjt(jt):
        for nm, w_d in (("wq", wq_d), ("wk", wk_d)):
            t = w_stage.tile([P, NT * P], F32, name=f"{nm}jt", tag="wjt")
            nc.sync.dma_start(
                out=t.rearrange("p (i j) -> p i j", j=P),
                in_=w_d.ap()
                .rearrange("(i p) d -> p i d", p=P)[:, :, jt * P : (jt + 1) * P],
            )
            wjt_f32[(nm, jt)] = t

    for st in range(NT):
        dma_x(st)
    emit_consts()
    dma_wjt(0)
    dma_wjt(1)
    wv_f32 = []
    for it in range(NT):
        t = wv_stage.tile([P, D], F32, name="wvf", tag="ws")
        nc.sync.dma_start(out=t, in_=wv_d.ap()[it * P : (it + 1) * P, :])
        wv_f32.append(t)
    for jt in range(2, NT):
        dma_wjt(jt)

    # ---- phase 1: X^T via PE transposes (fp32 in, bf16 out via DVE) ----
    xT = []
    for it in range(NT):
        xT.append(xT_pool.tile([P, S], BF16, name=f"xT{it}", tag=f"xT{it}"))

    def emit_xt(st_lo, st_hi):
        for st in range(st_lo, st_hi):
            for it in range(NT):
                pt = ps_proj.tile([P, P], F32, name="pt", tag="proj")
                nc.tensor.transpose(pt, x_t[st][:, it * P : (it + 1) * P], ident)
                nc.vector.tensor_copy(xT[it][:, st * P : (st + 1) * P], pt)

    # just-in-time weight conversions (DVE)
    w_bf = {}  # (nm, jt) -> [128, 1024] bf16 tile, it-major columns

    def cvt_wjt(jt):
        for nm in ("wq", "wk"):
            t = w_pool.tile([P, NT * P], BF16, name=f"{nm}b", tag=f"{nm}b{jt}")
            nc.vector.tensor_copy(t, wjt_f32[(nm, jt)])
            w_bf[(nm, jt)] = t

    wv_tiles = []

    def cvt_wv():
        for it in range(NT):
            t = w_pool.tile([P, D], BF16, name="wvb", tag=f"wvb{it}")
            nc.vector.tensor_copy(t, wv_f32[it])
            wv_tiles.append(t)

    # ---- V tiles (bf16, head-interleaved 65-col blocks w/ ones col) ----
    v_sb = []
    for st in range(NT):
        v = v_pool.tile([P, H * VW], BF16, name=f"v{st}", tag=f"v{st}")
        nc.gpsimd.memset(v, 1.0)  # ones columns survive at h*65+64
        v_sb.append(v)

    # bv broadcast across partitions (ones x bv rank-1, computed once) so
    # the per-st bias add rides the PSUM->SBUF copy instead of 16 matmuls.
    bvb = cst.tile([P, D], BF16, name="bvb", tag="bvb")

    def emit_bvb():
        mm = ps_sps.tile([P, S], F32, name="mmb", tag="sps")
        for jc in range(SC):
            nc.tensor.matmul(
                mm[:, jc * 512 : (jc + 1) * 512],
                lhsT=ones_row[0:1, 0:P],
                rhs=bv_row[0:1, jc * 512 : (jc + 1) * 512],
                start=True,
                stop=True,
            )
        nc.vector.tensor_copy(bvb, mm)

    def emit_v_tile(st):
        # per-jc [128,512] halves on the fast-turnover proj tag: V matmuls
        # never rotate the scores ring, so they don't wait on exps. Head
        # groups align: jc half = 8 head-blocks of 64 columns.
        vdst = v_sb[st].rearrange("p (g c) -> p g c", c=VW)
        bvb_g = bvb.rearrange("p (g c) -> p g c", c=DH)
        for jc in range(SC):
            mm = ps_proj.tile([P, 512], F32, name="mmv", tag="proj")
            for it in range(NT):
                nc.tensor.matmul(
                    mm,
                    lhsT=xT[it][:, st * P : (st + 1) * P],
                    rhs=wv_tiles[it][:, jc * 512 : (jc + 1) * 512],
                    start=(it == 0),
                    stop=(it == NT - 1),
                )
            nc.vector.scalar_tensor_tensor(
                out=vdst[:, jc * 8 : (jc + 1) * 8, 0:DH],
                in0=mm.rearrange("p (g c) -> p g c", c=DH),
                scalar=0.0,
                in1=bvb_g[:, jc * 8 : (jc + 1) * 8, :],
                op0=mybir.AluOpType.add,
                op1=mybir.AluOpType.add,
            )

    # ---- projections for one jt (Q^T and K^T rows jt*128..) ----
    qk_tiles = {}  # jt -> (qT, kT)

    def emit_proj(jt, scs=(0, 1)):
        if jt not in qk_tiles:
            qk_tiles[jt] = tuple(
                qk_pool.tile([P, S], BF16, name=f"{nm}T", tag="qk")
                for nm in ("bq", "bk")
            )
        for dst, nm, wkey in zip(qk_tiles[jt], ("bq", "bk"), ("wq", "wk")):
            # per-half psum tiles (own 1-bank tag: proj never waits on the
            # scores ring, so ACT keeps draining exps during projections)
            for sc in scs:
                mm = ps_proj.tile([P, 512], F32, name="mmp", tag="proj")
                for it in range(NT):
                    nc.tensor.matmul(
                        mm,
                        lhsT=w_bf[(wkey, jt)][:, it * P : (it + 1) * P],
                        rhs=xT[it][:, sc * 512 : (sc + 1) * 512],
                        start=(it == 0),
                        stop=(it == NT - 1),
                    )
                nc.vector.tensor_scalar_add(
                    dst[:, sc * 512 : (sc + 1) * 512],
                    mm,
                    b_cols[nm][:, jt : jt + 1],
                )

    # ---- scores + exp for one head (S^T[k,q] by kt tile) ----
    pT_store = {}  # h -> list of 8 pT tiles

    def emit_ctx_qt(h, pT, bounce, qt):
        cps = ps_ctx.tile([P, VW], F32, name="cps", tag="ctx")
        for kt in range(NT):
            nc.tensor.matmul(
                cps,
                lhsT=pT[kt][:, qt * P : (qt + 1) * P],
                rhs=v_sb[kt][:, h * VW : (h + 1) * VW],
                start=(kt == 0),
                stop=(kt == NT - 1),
            )
        r = small_pool.tile([P, 1], F32, name="recip", tag="recip")
        nc.vector.reciprocal(r, cps[:, DH : DH + 1])
        nc.vector.tensor_scalar_mul(
            bounce[:, qt * DH : (qt + 1) * DH], cps[:, 0:DH], r
        )

    def emit_ctx_out_dma(h, bounce):
        # one DMA per head: [p, qt, j] -> out[(qt p), h*64+j]
        nc.sync.dma_start(
            out=o_d.ap()
            .rearrange("(q p) d -> p q d", p=P)[:, :, h * DH : (h + 1) * DH],
            in_=bounce.rearrange("p (q j) -> p q j", j=DH),
        )

    def emit_scores_exp(h, ctx_heads=()):
        # scores+exp for head h, with the qt-blocks of lagged ctx heads
        # interleaved between kt-blocks: PE chews ctx matmuls while ACT
        # drains the scores psum ring instead of head-of-line stalling.
        jt, ro = h // 2, (h % 2) * DH
        qT_t, kT_t = qk_tiles[jt]
        jobs = [(c, pT_store.pop(c),
                 bounce_pool.tile([P, NT * DH], F32, name="bounce", tag="bounce"))
                for c in ctx_heads]
        pT = []
        for kt in range(NT):
            sps = ps_sps.tile([P, S], F32, name="sps", tag="sps")
            lhsT = kT_t[ro : ro + DH, kt * P : (kt + 1) * P]
            for qc in range(SC):
                nc.tensor.matmul(
                    sps[:, qc * 512 : (qc + 1) * 512],
                    lhsT=lhsT,
                    rhs=qT_t[ro : ro + DH, qc * 512 : (qc + 1) * 512],
                    start=True,
                    stop=True,
                )
            pt = p_pool.tile([P, S], BF16, name="pT", tag="pT")
            nc.scalar.activation(
                pt,
                sps,
                mybir.ActivationFunctionType.Exp,
                bias=nonlocal_store["mask_cols"][:, kt : kt + 1],
                scale=SCALE,
            )
            pT.append(pt)
            for c, cpT, bounce in jobs:
                emit_ctx_qt(c, cpT, bounce, kt)
        pT_store[h] = pT
        for c, cpT, bounce in jobs:
            emit_ctx_out_dma(c, bounce)

    # ---- standalone ctx (tail) ----
    def emit_ctx(h):
        pT = pT_store.pop(h)
        bounce = bounce_pool.tile([P, NT * DH], F32, name="bounce", tag="bounce")
        for qt in range(NT):
            emit_ctx_qt(h, pT, bounce, qt)
        emit_ctx_out_dma(h, bounce)

    # ---- pipelined schedule ----
    # ctx[h] lags scores/exp[h] by CTX_LAG heads mid-stream (pT buffering),
    # catching down to lag 1 at the tail so only ctx[15] runs after exp[15].
    next_ctx = [0]

    def emit_head(h):
        lag = min(CTX_LAG, max(1, H - 1 - h))
        ctx_heads = []
        while next_ctx[0] <= h - lag:
            ctx_heads.append(next_ctx[0])
            next_ctx[0] += 1
        emit_scores_exp(h, ctx_heads)

    emit_xt(0, NT)
    cvt_wjt(0)
    emit_proj(0)
    cvt_wjt(1)
    emit_head(0)
    cvt_wjt(2)
    emit_head(1)
    cvt_wv()
    emit_bvb()
    emit_proj(1)
    # V interleaved with heads 2,3: PE chews V while ACT drains the exp
    # backlog of heads 0,1; all of V lands before ctx[0] (h=3).
    for st in range(0, 4):
        emit_v_tile(st)
    emit_head(2)
    for st in range(4, NT):
        emit_v_tile(st)
    emit_head(3)
    for jt in range(2, NT):
        emit_proj(jt)
        if jt + 1 < NT:
            cvt_wjt(jt + 1)
        emit_head(2 * jt)
        emit_head(2 * jt + 1)
    while next_ctx[0] < H:
        emit_ctx(next_ctx[0])
        next_ctx[0] += 1


def build_program(n_reps: int = 1, n_loop: int = 0) -> bass.Bass:
    nc = bacc.Bacc(trn_type="TRN2", target_bir_lowering=False, debug=False)

    x_d = nc.declare_dram_parameter("hidden_states", [S, D], F32, isOutput=False)
    m_d = nc.declare_dram_parameter("attention_mask", [S], F32, isOutput=False)
    wq_d = nc.declare_dram_parameter("Wq", [D, D], F32, isOutput=False)
    bq_d = nc.declare_dram_parameter("bq", [D], F32, isOutput=False)
    wk_d = nc.declare_dram_parameter("Wk", [D, D], F32, isOutput=False)
    bk_d = nc.declare_dram_parameter("bk", [D], F32, isOutput=False)
    wv_d = nc.declare_dram_parameter("Wv", [D, D], F32, isOutput=False)
    bv_d = nc.declare_dram_parameter("bv", [D], F32, isOutput=False)
    o_d = nc.declare_dram_parameter("out", [S, D], F32, isOutput=True)
    dram = (x_d, m_d, wq_d, bq_d, wk_d, bk_d, wv_d, bv_d, o_d)

    with tile.TileContext(nc) as tc:
        with (
            tc.tile_pool(name="consts", bufs=1) as cst,
            tc.tile_pool(name="xT", bufs=1) as xT_pool,
            tc.tile_pool(name="wpool", bufs=1) as w_pool,
            tc.tile_pool(name="qk", bufs=4) as qk_pool,
            tc.tile_pool(name="vsb", bufs=1) as v_pool,
            tc.tile_pool(name="xstage", bufs=2) as x_stage,
            # [128,1024]f32 rings: 1.5 jt's worth of Wq+Wk column-block
            # prefetch; Wv full rows get their own ring.
            tc.tile_pool(name="wstage", bufs=2) as w_stage,
            tc.tile_pool(name="wvstage", bufs=2) as wv_stage,
            tc.tile_pool(name="pT", bufs=8 * (CTX_LAG + 1)) as p_pool,
            tc.tile_pool(name="small", bufs=16) as small_pool,
            tc.tile_pool(name="bounce", bufs=2) as bounce_pool,
            # PSUM (8 banks): scores [128,1024] x2 = 4, proj [128,512] x2
            # = 2, ctx [128,65] x2 = 2.
            tc.tile_pool(name="pssps", bufs=2, space="PSUM") as ps_sps,
            tc.tile_pool(name="psproj", bufs=2, space="PSUM") as ps_proj,
            tc.tile_pool(name="psctx", bufs=2, space="PSUM") as ps_ctx,
        ):
            ident = cst.tile([P, P], F32, name="ident", tag="ident")
            make_identity(nc, ident)
            pools = (cst, xT_pool, w_pool, qk_pool, v_pool, x_stage, w_stage,
                     wv_stage, p_pool, small_pool, bounce_pool, ps_sps,
                     ps_proj, ps_ctx, ident)
            if n_loop:
                with tc.For_i(0, n_loop, 1):
                    emit_body(nc, dram, pools)
            else:
                for _ in range(n_reps):
                    emit_body(nc, dram, pools)
    nc.compile()
    return nc


_NC_CACHE = None


def _get_nc():
    global _NC_CACHE
    if _NC_CACHE is None:
        _NC_CACHE = build_program()
    return _NC_CACHE


def make_in_maps(hidden_states, attention_mask, Wq, bq, Wk, bk, Wv, bv):
    hs = np.ascontiguousarray(np.asarray(hidden_states, dtype=np.float32))
    am = np.ascontiguousarray(
        np.asarray(attention_mask, dtype=np.float32).reshape(B, S)
    )
    shared = {
        "Wq": np.ascontiguousarray(np.asarray(Wq, dtype=np.float32)),
        "bq": np.ascontiguousarray(np.asarray(bq, dtype=np.float32)),
        "Wk": np.ascontiguousarray(np.asarray(Wk, dtype=np.float32)),
        "bk": np.ascontiguousarray(np.asarray(bk, dtype=np.float32)),
        "Wv": np.ascontiguousarray(np.asarray(Wv, dtype=np.float32)),
        "bv": np.ascontiguousarray(np.asarray(bv, dtype=np.float32)),
    }
    return [
        {"hidden_states": hs[b], "attention_mask": am[b], **shared}
        for b in range(B)
    ]


def kernel(hidden_states, attention_mask, Wq, bq, Wk, bk, Wv, bv):
    nc = _get_nc()
    in_maps = make_in_maps(hidden_states, attention_mask, Wq, bq, Wk, bk, Wv, bv)
    res = run_bass_kernel_spmd(nc, in_maps, list(range(N_CORES))).results
    out = np.stack([np.asarray(res[b]["out"], dtype=np.float32) for b in range(B)])
    return out


# revision 80
# speedup vs baseline: 1.0307x; 1.0256x over previous
"""BERT self-attention (B=8, S=1024, D=1024, H=16, DH=64) on 8 Trainium2 cores.

Strategy: pure data-parallel over batch - each of the 8 cores runs the full
self-attention for one batch element. No collectives.

Single software-pipelined stream so the ACT engine's exp work (~134us/core,
1 elem/lane/cycle, irreducible) hides under the PE's matmul work instead of
serializing after it (sim: PE and ACT both 99-100% busy in steady state):

  X^T (PE transposes of fp32 X, psum->sbuf copies convert to bf16)
  jt=0: Q^T/K^T proj -> scores+exp heads 0,1
  jt=1: proj -> V tiles interleaved with scores+exp heads 2,3
  jt=2..7: proj -> scores+exp heads 2jt,2jt+1, with the qt-blocks of
           ctx[h-3] interleaved between scores kt-blocks; the lag catches
           down to 1 at the tail so only ctx[15] runs after exp[15]

Key scheduling decisions (each HW-measured):
  - Wq/Wk arrive as per-jt column blocks (one 3D-AP DMA each) so proj jt=0
    needs just 1MB of W beyond X: the startup is DMA-roofline-bound (16MB
    of fp32 inputs) and exp starts ~20us earlier than whole-matrix
    delivery. DMA instruction count is kept low (flat HWDGE cost/DMA):
    one DMA per W column-block, one output DMA per head.
  - Projections and V accumulate in their own 1-bank [128,512] PSUM tag.
    Sharing the scores ring made them wait on exp completions (engine
    FIFOs + 2-deep psum ring), starving ACT during projections: -19us HW.
  - PSUM (8 banks): scores [128,1024]x2, proj/V [128,512]x2, ctx
    [128,65]x2.
  - V bias rides the PSUM->SBUF copy (scalar_tensor_tensor add of a
    precomputed ones x bv broadcast): -27us HW vs rank-1 bias matmuls that
    broke the PSUM accumulation groups.

Datapath is bf16 end-to-end on the PE (weights, X^T, Q^T, K^T, V, probs):
1 cycle/row matmuls everywhere plus fast weight loads (FWL needs non-fp32
dtype). PSUM accumulation stays fp32; rel err vs fp32 reference ~3.1e-3.
W/X arrive fp32 via DMA and are converted on the DVE. Q/K biases are folded
into the PSUM->SBUF copy as per-partition tensor_scalar adds (j sits on
partitions in Q^T/K^T). The attention mask (indexed by k) is a
per-partition bias folded with the 1/sqrt(DH) scale into the Exp activation
on transposed scores S^T[k,q]. The V tiles carry a ones column per head so
the context matmul emits the softmax denominator for free; DVE reciprocal +
tensor_scalar multiply normalize, writing a per-head bounce tile that goes
out in a single DMA.

Built on bacc.Bacc: its compile() legalizes sync waits (1 wait/instruction
hardware limit) via move_matmul_waits_to_ldweights + generate_event_semaphores.
"""

import numpy as np

import concourse.bass as bass
import concourse.bacc as bacc
import concourse.mybir as mybir
import concourse.tile as tile
from concourse.bass_utils import run_bass_kernel_spmd
from concourse.masks import make_identity

F32 = mybir.dt.float32
F32R = mybir.dt.float32r
BF16 = mybir.dt.bfloat16

B, S, D, H = 8, 1024, 1024, 16
DH = D // H  # 64
P = 128
NT = S // P  # 8 tiles along any 1024 dim
SC = S // 512  # 2 chunks of 512
SCALE = 1.0 / float(np.sqrt(DH))
N_CORES = 8
VW = DH + 1  # 65: V block width per head (64 cols + ones col)
CTX_LAG = 3  # ctx[h-CTX_LAG] is emitted after scores/exp[h]


def emit_body(nc, dram, pools):
    (x_d, m_d, wq_d, bq_d, wk_d, bk_d, wv_d, bv_d, o_d) = dram
    (cst, xT_pool, w_pool, qk_pool, v_pool, x_stage, w_stage, wv_stage,
     p_pool, small_pool, bounce_pool, ps_sps, ps_proj, ps_ctx, ident) = pools

    # ---- input DMAs (front-loaded so the DGE rings start immediately;
    # staging pool depths pace them) ----
    x_t = []

    def dma_x(st):
        t = x_stage.tile([P, D], F32, name="x_tile", tag="xs")
        nc.sync.dma_start(out=t, in_=x_d.ap()[st * P : (st + 1) * P, :])
        x_t.append(t)

    nonlocal_store = {}
    b_cols = {}

    def emit_consts():
        # tiny gathers; after X so x0 isn't delayed, before the W streams
        # so the bias/mask consumers aren't starved
        m = cst.tile([P, NT], F32, name="mask_cols", tag="mask_cols")
        nc.sync.dma_start(out=m, in_=m_d.ap().rearrange("(g p) -> p g", p=P))
        nonlocal_store["mask_cols"] = m
        for nm, hd in (("bq", bq_d), ("bk", bk_d)):
            t = cst.tile([P, NT], F32, name=f"bcol_{nm}", tag=f"bcol_{nm}")
            nc.sync.dma_start(out=t, in_=hd.ap().rearrange("(g p) -> p g", p=P))
            b_cols[nm] = t
        bv_f32 = cst.tile([1, D], F32, name="bv_f32", tag="bv_f32")
        nc.sync.dma_start(out=bv_f32, in_=bv_d.ap().unsqueeze(0))
        nc.vector.tensor_copy(bv_row, bv_f32)

    bv_row = cst.tile([1, D], BF16, name="bv_row", tag="bv_row")
    ones_f32 = cst.tile([1, P], F32, name="ones_f32", tag="ones_f32")
    nc.vector.memset(ones_f32, 1.0)
    ones_row = cst.tile([1, P], BF16, name="ones_row", tag="ones_row")
    nc.vector.tensor_copy(ones_row, ones_f32)

    # Wq/Wk arrive as per-jt column blocks (one [128, it*128] DMA each) so
    # projection jt needs only 1MB of W beyond X - scores/exp start ~20us
    # earlier than with whole-matrix delivery. Wv arrives as full rows (V
    # streams them as rhs).
    wjt_f32 = {}  # (nm, jt) -> [128, 1024] f32 tile, it-major columns

    def dma_wjt(jt):
        for nm, w_d in (("wq", wq_d), ("wk", wk_d)):
            t = w_stage.tile([P, NT * P], F32, name=f"{nm}jt", tag="wjt")
            nc.sync.dma_start(
                out=t.rearrange("p (i j) -> p i j", j=P),
                in_=w_d.ap()
                .rearrange("(i p) d -> p i d", p=P)[:, :, jt * P : (jt + 1) * P],
            )
            wjt_f32[(nm, jt)] = t

    for st in range(NT):
        dma_x(st)
    emit_consts()
    dma_wjt(0)
    dma_wjt(1)
    wv_f32 = []
    for it in range(NT):
        t = wv_stage.tile([P, D], F32, name="wvf", tag="ws")
        nc.sync.dma_start(out=t, in_=wv_d.ap()[it * P : (it + 1) * P, :])
        wv_f32.append(t)
    for jt in range(2, NT):
        dma_wjt(jt)

    # ---- phase 1: X^T via PE transposes (fp32 in, bf16 out via DVE) ----
    xT = []
    for it in range(NT):
        xT.append(xT_pool.tile([P, S], BF16, name=f"xT{it}", tag=f"xT{it}"))

    def emit_xt(st_lo, st_hi):
        for st in range(st_lo, st_hi):
            for it in range(NT):
                pt = ps_proj.tile([P, P], F32, name="pt", tag="proj")
                nc.tensor.transpose(pt, x_t[st][:, it * P : (it + 1) * P], ident)
                nc.vector.tensor_copy(xT[it][:, st * P : (st + 1) * P], pt)

    # just-in-time weight conversions (DVE)
    w_bf = {}  # (nm, jt) -> [128, 1024] bf16 tile, it-major columns

    def cvt_wjt(jt):
        for nm in ("wq", "wk"):
            t = w_pool.tile([P, NT * P], BF16, name=f"{nm}b", tag=f"{nm}b{jt}")
            nc.vector.tensor_copy(t, wjt_f32[(nm, jt)])
            w_bf[(nm, jt)] = t

    wv_tiles = []

    def cvt_wv():
        for it in range(NT):
            t = w_pool.tile([P, D], BF16, name="wvb", tag=f"wvb{it}")
            nc.vector.tensor_copy(t, wv_f32[it])
            wv_tiles.append(t)

    # ---- V tiles (bf16, head-interleaved 65-col blocks w/ ones col) ----
    v_sb = []
    for st in range(NT):
        v = v_pool.tile([P, H * VW], BF16, name=f"v{st}", tag=f"v{st}")
        nc.gpsimd.memset(v, 1.0)  # ones columns survive at h*65+64
        v_sb.append(v)

    # bv broadcast across partitions (ones x bv rank-1, computed once) so
    # the per-st bias add rides the PSUM->SBUF copy instead of 16 matmuls.
    bvb = cst.tile([P, D], BF16, name="bvb", tag="bvb")

    def emit_bvb():
        mm = ps_sps.tile([P, S], F32, name="mmb", tag="sps")
        for jc in range(SC):
            nc.tensor.matmul(
                mm[:, jc * 512 : (jc + 1) * 512],
                lhsT=ones_row[0:1, 0:P],
                rhs=bv_row[0:1, jc * 512 : (jc + 1) * 512],
                start=True,
                stop=True,
            )
        nc.vector.tensor_copy(bvb, mm)

    def emit_v_tile(st):
        # per-jc [128,512] halves on the fast-turnover proj tag: V matmuls
        # never rotate the scores ring, so they don't wait on exps. Head
        # groups align: jc half = 8 head-blocks of 64 columns.
        vdst = v_sb[st].rearrange("p (g c) -> p g c", c=VW)
        bvb_g = bvb.rearrange("p (g c) -> p g c", c=DH)
        for jc in range(SC):
            mm = ps_proj.tile([P, 512], F32, name="mmv", tag="proj")
            for it in range(NT):
                nc.tensor.matmul(
                    mm,
                    lhsT=xT[it][:, st * P : (st + 1) * P],
                    rhs=wv_tiles[it][:, jc * 512 : (jc + 1) * 512],
                    start=(it == 0),
                    stop=(it == NT - 1),
                )
            nc.vector.scalar_tensor_tensor(
                out=vdst[:, jc * 8 : (jc + 1) * 8, 0:DH],
                in0=mm.rearrange("p (g c) -> p g c", c=DH),
                scalar=0.0,
                in1=bvb_g[:, jc * 8 : (jc + 1) * 8, :],
                op0=mybir.AluOpType.add,
                op1=mybir.AluOpType.add,
            )

    # ---- projections for one jt (Q^T and K^T rows jt*128..) ----
    qk_tiles = {}  # jt -> (qT, kT)

    def emit_proj(jt, scs=(0, 1)):
        if jt not in qk_tiles:
            qk_tiles[jt] = tuple(
                qk_pool.tile([P, S], BF16, name=f"{nm}T", tag="qk")
                for nm in ("bq", "bk")
            )
        for dst, nm, wkey in zip(qk_tiles[jt], ("bq", "bk"), ("wq", "wk")):
            # per-half psum tiles (own 1-bank tag: proj never waits on the
            # scores ring, so ACT keeps draining exps during projections)
            for sc in scs:
                mm = ps_proj.tile([P, 512], F32, name="mmp", tag="proj")
                for it in range(NT):
                    nc.tensor.matmul(
                        mm,
                        lhsT=w_bf[(wkey, jt)][:, it * P : (it + 1) * P],
                        rhs=xT[it][:, sc * 512 : (sc + 1) * 512],
                        start=(it == 0),
                        stop=(it == NT - 1),
                    )
                nc.vector.tensor_scalar_add(
                    dst[:, sc * 512 : (sc + 1) * 512],
                    mm,
                    b_cols[nm][:, jt : jt + 1],
                )

    # ---- scores + exp for one head (S^T[k,q] by kt tile) ----
    pT_store = {}  # h -> list of 8 pT tiles

    def emit_ctx_qt(h, pT, bounce, qt):
        cps = ps_ctx.tile([P, VW], F32, name="cps", tag="ctx")
        for kt in range(NT):
            nc.tensor.matmul(
                cps,
                lhsT=pT[kt][:, qt * P : (qt + 1) * P],
                rhs=v_sb[kt][:, h * VW : (h + 1) * VW],
                start=(kt == 0),
                stop=(kt == NT - 1),
            )
        r = small_pool.tile([P, 1], F32, name="recip", tag="recip")
        nc.vector.reciprocal(r, cps[:, DH : DH + 1])
        nc.vector.tensor_scalar_mul(
            bounce[:, qt * DH : (qt + 1) * DH], cps[:, 0:DH], r
        )

    def emit_ctx_out_dma(h, bounce):
        # one DMA per head: [p, qt, j] -> out[(qt p), h*64+j]
        nc.sync.dma_start(
            out=o_d.ap()
            .rearrange("(q p) d -> p q d", p=P)[:, :, h * DH : (h + 1) * DH],
            in_=bounce.rearrange("p (q j) -> p q j", j=DH),
        )

    def emit_scores_exp(h, ctx_heads=()):
        # scores+exp for head h, with the qt-blocks of lagged ctx heads
        # interleaved between kt-blocks: PE chews ctx matmuls while ACT
        # drains the scores psum ring instead of head-of-line stalling.
        jt, ro = h // 2, (h % 2) * DH
        qT_t, kT_t = qk_tiles[jt]
        jobs = [(c, pT_store.pop(c),
                 bounce_pool.tile([P, NT * DH], F32, name="bounce", tag="bounce"))
                for c in ctx_heads]
        pT = []
        for kt in range(NT):
            sps = ps_sps.tile([P, S], F32, name="sps", tag="sps")
            lhsT = kT_t[ro : ro + DH, kt * P : (kt + 1) * P]
            for qc in range(SC):
                nc.tensor.matmul(
                    sps[:, qc * 512 : (qc + 1) * 512],
                    lhsT=lhsT,
                    rhs=qT_t[ro : ro + DH, qc * 512 : (qc + 1) * 512],
                    start=True,
                    stop=True,
                )
            pt = p_pool.tile([P, S], BF16, name="pT", tag="pT")
            nc.scalar.activation(
                pt,
                sps,
                mybir.ActivationFunctionType.Exp,
                bias=nonlocal_store["mask_cols"][:, kt : kt + 1],
                scale=SCALE,
            )
            pT.append(pt)
            for c, cpT, bounce in jobs:
                emit_ctx_qt(c, cpT, bounce, kt)
        pT_store[h] = pT
        for c, cpT, bounce in jobs:
            emit_ctx_out_dma(c, bounce)

    # ---- standalone ctx (tail) ----
    def emit_ctx(h):
        pT = pT_store.pop(h)
        bounce = bounce_pool.tile([P, NT * DH], F32, name="bounce", tag="bounce")
        for qt in range(NT):
            emit_ctx_qt(h, pT, bounce, qt)
        emit_ctx_out_dma(h, bounce)

    # ---- pipelined schedule ----
    # ctx[h] lags scores/exp[h] by CTX_LAG heads mid-stream (pT buffering),
    # catching down to lag 1 at the tail so only ctx[15] runs after exp[15].
    next_ctx = [0]

    def emit_head(h):
        lag = min(CTX_LAG, max(1, H - 1 - h))
        ctx_heads = []
        while next_ctx[0] <= h - lag:
            ctx_heads.append(next_ctx[0])
            next_ctx[0] += 1
        emit_scores_exp(h, ctx_heads)

    emit_xt(0, NT)
    cvt_wjt(0)
    emit_proj(0)
    cvt_wjt(1)
    emit_head(0)
    cvt_wjt(2)
    emit_head(1)
    cvt_wv()
    emit_bvb()
    emit_proj(1)
    # V interleaved with heads 2,3: PE chews V while ACT drains the exp
    # backlog of heads 0,1; all of V lands before ctx[0] (h=3).
    for st in range(0, 4):
        emit_v_tile(st)
    emit_head(2)
    for st in range(4, NT):
        emit_v_tile(st)
    emit_head(3)
    for jt in range(2, NT):
        emit_proj(jt)
        if jt + 1 < NT:
            cvt_wjt(jt + 1)
        emit_head(2 * jt)
        emit_head(2 * jt + 1)
    while next_ctx[0] < H:
        emit_ctx(next_ctx[0])
        next_ctx[0] += 1


def build_program(n_reps: int = 1, n_loop: int = 0) -> bass.Bass:
    nc = bacc.Bacc(trn_type="TRN2", target_bir_lowering=False, debug=False)

    x_d = nc.declare_dram_parameter("hidden_states", [S, D], F32, isOutput=False)
    m_d = nc.declare_dram_parameter("attention_mask", [S], F32, isOutput=False)
    wq_d = nc.declare_dram_parameter("Wq", [D, D], F32, isOutput=False)
    bq_d = nc.declare_dram_parameter("bq", [D], F32, isOutput=False)
    wk_d = nc.declare_dram_parameter("Wk", [D, D], F32, isOutput=False)
    bk_d = nc.declare_dram_parameter("bk", [D], F32, isOutput=False)
    wv_d = nc.declare_dram_parameter("Wv", [D, D], F32, isOutput=False)
    bv_d = nc.declare_dram_parameter("bv", [D], F32, isOutput=False)
    o_d = nc.declare_dram_parameter("out", [S, D], F32, isOutput=True)
    dram = (x_d, m_d, wq_d, bq_d, wk_d, bk_d, wv_d, bv_d, o_d)

    with tile.TileContext(nc) as tc:
        with (
            tc.tile_pool(name="consts", bufs=1) as cst,
            tc.tile_pool(name="xT", bufs=1) as xT_pool,
            tc.tile_pool(name="wpool", bufs=1) as w_pool,
            tc.tile_pool(name="qk", bufs=4) as qk_pool,
            tc.tile_pool(name="vsb", bufs=1) as v_pool,
            tc.tile_pool(name="xstage", bufs=2) as x_stage,
            # [128,1024]f32 rings: 1.5 jt's worth of Wq+Wk column-block
            # prefetch; Wv full rows get their own ring.
            tc.tile_pool(name="wstage", bufs=2) as w_stage,
            tc.tile_pool(name="wvstage", bufs=2) as wv_stage,
            tc.tile_pool(name="pT", bufs=8 * (CTX_LAG + 1)) as p_pool,
            tc.tile_pool(name="small", bufs=16) as small_pool,
            tc.tile_pool(name="bounce", bufs=2) as bounce_pool,
            # PSUM (8 banks): scores [128,1024] x2 = 4, proj [128,512] x2
            # = 2, ctx [128,65] x2 = 2.
            tc.tile_pool(name="pssps", bufs=2, space="PSUM") as ps_sps,
            tc.tile_pool(name="psproj", bufs=2, space="PSUM") as ps_proj,
            tc.tile_pool(name="psctx", bufs=2, space="PSUM") as ps_ctx,
        ):
            ident = cst.tile([P, P], F32, name="ident", tag="ident")
            make_identity(nc, ident)
            pools = (cst, xT_pool, w_pool, qk_pool, v_pool, x_stage, w_stage,
                     wv_stage, p_pool, small_pool, bounce_pool, ps_sps,
                     ps_proj, ps_ctx, ident)
            if n_loop:
                with tc.For_i(0, n_loop, 1):
                    emit_body(nc, dram, pools)
            else:
                for _ in range(n_reps):
                    emit_body(nc, dram, pools)
    nc.compile()
    return nc


_NC_CACHE = None


def _get_nc():
    global _NC_CACHE
    if _NC_CACHE is None:
        _NC_CACHE = build_program()
    return _NC_CACHE


def make_in_maps(hidden_states, attention_mask, Wq, bq, Wk, bk, Wv, bv):
    hs = np.ascontiguousarray(np.asarray(hidden_states, dtype=np.float32))
    am = np.ascontiguousarray(
        np.asarray(attention_mask, dtype=np.float32).reshape(B, S)
    )
    shared = {
        "Wq": np.ascontiguousarray(np.asarray(Wq, dtype=np.float32)),
        "bq": np.ascontiguousarray(np.asarray(bq, dtype=np.float32)),
        "Wk": np.ascontiguousarray(np.asarray(Wk, dtype=np.float32)),
        "bk": np.ascontiguousarray(np.asarray(bk, dtype=np.float32)),
        "Wv": np.ascontiguousarray(np.asarray(Wv, dtype=np.float32)),
        "bv": np.ascontiguousarray(np.asarray(bv, dtype=np.float32)),
    }
    return [
        {"hidden_states": hs[b], "attention_mask": am[b], **shared}
        for b in range(B)
    ]


def kernel(hidden_states, attention_mask, Wq, bq, Wk, bk, Wv, bv):
    nc = _get_nc()
    in_maps = make_in_maps(hidden_states, attention_mask, Wq, bq, Wk, bk, Wv, bv)
    res = run_bass_kernel_spmd(nc, in_maps, list(range(N_CORES))).results
    out = np.stack([np.asarray(res[b]["out"], dtype=np.float32) for b in range(B)])
    return out


# revision 83
# speedup vs baseline: 1.0349x; 1.0041x over previous
"""BERT self-attention (B=8, S=1024, D=1024, H=16, DH=64) on 8 Trainium2 cores.

Strategy: pure data-parallel over batch - each of the 8 cores runs the full
self-attention for one batch element. No collectives.

Single software-pipelined stream so the ACT engine's exp work (~134us/core,
1 elem/lane/cycle, irreducible) hides under the PE's matmul work instead of
serializing after it (sim: PE and ACT both 99-100% busy in steady state):

  X^T (PE transposes of fp32 X, psum->sbuf copies convert to bf16)
  jt=0: Q^T/K^T proj -> scores+exp heads 0,1
  jt=1: proj -> V tiles interleaved with scores+exp heads 2,3
  jt=2..7: proj -> scores+exp heads 2jt,2jt+1, with the qt-blocks of
           ctx[h-3] interleaved between scores kt-blocks; the lag catches
           down to 1 at the tail so only ctx[15] runs after exp[15]

Key scheduling decisions (each HW-measured):
  - Wq/Wk arrive as per-jt column blocks (one 3D-AP DMA each) so proj jt=0
    needs just 1MB of W beyond X: the startup is DMA-roofline-bound (16MB
    of fp32 inputs) and exp starts ~20us earlier than whole-matrix
    delivery. DMA instruction count is kept low (flat HWDGE cost/DMA):
    one DMA per W column-block, one output DMA per head.
  - Projections and V accumulate in their own 1-bank [128,512] PSUM tag.
    Sharing the scores ring made them wait on exp completions (engine
    FIFOs + 2-deep psum ring), starving ACT during projections: -19us HW.
  - PSUM (8 banks): scores [128,1024]x2, proj/V [128,512]x2, ctx
    [128,65]x2.
  - V bias rides the PSUM->SBUF copy (scalar_tensor_tensor add of a
    precomputed ones x bv broadcast): -27us HW vs rank-1 bias matmuls that
    broke the PSUM accumulation groups.

Datapath is bf16 end-to-end on the PE (weights, X^T, Q^T, K^T, V, probs):
1 cycle/row matmuls everywhere plus fast weight loads (FWL needs non-fp32
dtype). PSUM accumulation stays fp32; rel err vs fp32 reference ~3.1e-3.
W/X arrive fp32 via DMA and are converted on the DVE. Q/K biases are folded
into the PSUM->SBUF copy as per-partition tensor_scalar adds (j sits on
partitions in Q^T/K^T). The attention mask (indexed by k) is a
per-partition bias folded with the 1/sqrt(DH) scale into the Exp activation
on transposed scores S^T[k,q]. The V tiles carry a ones column per head so
the context matmul emits the softmax denominator for free; DVE reciprocal +
tensor_scalar multiply normalize, writing a per-head bounce tile that goes
out in a single DMA.

Built on bacc.Bacc: its compile() legalizes sync waits (1 wait/instruction
hardware limit) via move_matmul_waits_to_ldweights + generate_event_semaphores.
"""

import numpy as np

import concourse.bass as bass
import concourse.bacc as bacc
import concourse.mybir as mybir
import concourse.tile as tile
from concourse.bass_utils import run_bass_kernel_spmd
from concourse.masks import make_identity

F32 = mybir.dt.float32
F32R = mybir.dt.float32r
BF16 = mybir.dt.bfloat16

B, S, D, H = 8, 1024, 1024, 16
DH = D // H  # 64
P = 128
NT = S // P  # 8 tiles along any 1024 dim
SC = S // 512  # 2 chunks of 512
SCALE = 1.0 / float(np.sqrt(DH))
N_CORES = 8
VW = DH + 1  # 65: V block width per head (64 cols + ones col)
CTX_LAG = 3  # ctx[h-CTX_LAG] is emitted after scores/exp[h]


def emit_body(nc, dram, pools):
    (x_d, m_d, wq_d, bq_d, wk_d, bk_d, wv_d, bv_d, o_d) = dram
    (cst, xT_pool, w_pool, qk_pool, v_pool, x_stage, w_stage, wv_stage,
     p_pool, small_pool, bounce_pool, ps_sps, ps_proj, ps_ctx, ident) = pools

    # ---- input DMAs (front-loaded so the DGE rings start immediately;
    # staging pool depths pace them) ----
    x_t = []

    def dma_x(st):
        t = x_stage.tile([P, D], F32, name="x_tile", tag="xs")
        nc.sync.dma_start(out=t, in_=x_d.ap()[st * P : (st + 1) * P, :])
        x_t.append(t)

    nonlocal_store = {}
    b_cols = {}

    def emit_consts():
        # tiny gathers; after X so x0 isn't delayed, before the W streams
        # so the bias/mask consumers aren't starved
        m = cst.tile([P, NT], F32, name="mask_cols", tag="mask_cols")
        nc.sync.dma_start(out=m, in_=m_d.ap().rearrange("(g p) -> p g", p=P))
        nonlocal_store["mask_cols"] = m
        for nm, hd in (("bq", bq_d), ("bk", bk_d)):
            t = cst.tile([P, NT], F32, name=f"bcol_{nm}", tag=f"bcol_{nm}")
            nc.sync.dma_start(out=t, in_=hd.ap().rearrange("(g p) -> p g", p=P))
            b_cols[nm] = t
        bv_f32 = cst.tile([1, D], F32, name="bv_f32", tag="bv_f32")
        nc.sync.dma_start(out=bv_f32, in_=bv_d.ap().unsqueeze(0))
        nc.vector.tensor_copy(bv_row, bv_f32)

    bv_row = cst.tile([1, D], BF16, name="bv_row", tag="bv_row")
    ones_f32 = cst.tile([1, P], F32, name="ones_f32", tag="ones_f32")
    nc.vector.memset(ones_f32, 1.0)
    ones_row = cst.tile([1, P], BF16, name="ones_row", tag="ones_row")
    nc.vector.tensor_copy(ones_row, ones_f32)

    # Wq/Wk arrive as per-jt column blocks (one [128, it*128] DMA each) so
    # projection jt needs only 1MB of W beyond X - scores/exp start ~20us
    # earlier than with whole-matrix delivery. Wv arrives as full rows (V
    # streams them as rhs).
    wjt_f32 = {}  # (nm, jt) -> [128, 1024] f32 tile, it-major columns

    def dma_wjt(jt):
        for nm, w_d in (("wq", wq_d), ("wk", wk_d)):
            t = w_stage.tile([P, NT * P], F32, name=f"{nm}jt", tag="wjt")
            nc.sync.dma_start(
                out=t.rearrange("p (i j) -> p i j", j=P),
                in_=w_d.ap()
                .rearrange("(i p) d -> p i d", p=P)[:, :, jt * P : (jt + 1) * P],
            )
            wjt_f32[(nm, jt)] = t

    for st in range(NT):
        dma_x(st)
    emit_consts()
    dma_wjt(0)
    dma_wjt(1)
    wv_f32 = []
    for it in range(NT):
        t = wv_stage.tile([P, D], F32, name="wvf", tag="ws")
        nc.sync.dma_start(out=t, in_=wv_d.ap()[it * P : (it + 1) * P, :])
        wv_f32.append(t)
    for jt in range(2, NT):
        dma_wjt(jt)

    # ---- phase 1: X^T via PE transposes (fp32 in, bf16 out via DVE) ----
    # One [128,1024] psum tile (sps tag - idle during X^T) takes all 8
    # transposes of an st row-block, drained by ONE strided DVE copy into
    # the unified xT tile: 8 copies instead of 64, so the phase is paced
    # by the X DMA (~14us) instead of per-copy latency (~31us).
    xT_all = xT_pool.tile([P, NT * S], BF16, name="xT", tag="xT")

    def xTs(it):
        return xT_all[:, it * S : (it + 1) * S]

    def emit_xt(st_lo, st_hi):
        for st in range(st_lo, st_hi):
            pt = ps_sps.tile([P, S], F32, name="pt", tag="sps")
            for it in range(NT):
                nc.tensor.transpose(
                    pt[:, it * P : (it + 1) * P],
                    x_t[st][:, it * P : (it + 1) * P],
                    ident,
                )
            nc.vector.tensor_copy(
                xT_all.rearrange("p (i c) -> p i c", c=S)[
                    :, :, st * P : (st + 1) * P
                ],
                pt.rearrange("p (i c) -> p i c", c=P),
            )

    # just-in-time weight conversions (DVE)
    w_bf = {}  # (nm, jt) -> [128, 1024] bf16 tile, it-major columns

    def cvt_wjt(jt):
        for nm in ("wq", "wk"):
            t = w_pool.tile([P, NT * P], BF16, name=f"{nm}b", tag=f"{nm}b{jt}")
            nc.vector.tensor_copy(t, wjt_f32[(nm, jt)])
            w_bf[(nm, jt)] = t

    wv_tiles = []

    def cvt_wv():
        for it in range(NT):
            t = w_pool.tile([P, D], BF16, name="wvb", tag=f"wvb{it}")
            nc.vector.tensor_copy(t, wv_f32[it])
            wv_tiles.append(t)

    # ---- V tiles (bf16, head-interleaved 65-col blocks w/ ones col) ----
    v_sb = []
    for st in range(NT):
        v = v_pool.tile([P, H * VW], BF16, name=f"v{st}", tag=f"v{st}")
        nc.gpsimd.memset(v, 1.0)  # ones columns survive at h*65+64
        v_sb.append(v)

    # bv broadcast across partitions (ones x bv rank-1, computed once) so
    # the per-st bias add rides the PSUM->SBUF copy instead of 16 matmuls.
    bvb = cst.tile([P, D], BF16, name="bvb", tag="bvb")

    def emit_bvb():
        mm = ps_sps.tile([P, S], F32, name="mmb", tag="sps")
        for jc in range(SC):
            nc.tensor.matmul(
                mm[:, jc * 512 : (jc + 1) * 512],
                lhsT=ones_row[0:1, 0:P],
                rhs=bv_row[0:1, jc * 512 : (jc + 1) * 512],
                start=True,
                stop=True,
            )
        nc.vector.tensor_copy(bvb, mm)

    def emit_v_tile(st):
        # per-jc [128,512] halves on the fast-turnover proj tag: V matmuls
        # never rotate the scores ring, so they don't wait on exps. Head
        # groups align: jc half = 8 head-blocks of 64 columns.
        vdst = v_sb[st].rearrange("p (g c) -> p g c", c=VW)
        bvb_g = bvb.rearrange("p (g c) -> p g c", c=DH)
        for jc in range(SC):
            mm = ps_proj.tile([P, 512], F32, name="mmv", tag="proj")
            for it in range(NT):
                nc.tensor.matmul(
                    mm,
                    lhsT=xTs(it)[:, st * P : (st + 1) * P],
                    rhs=wv_tiles[it][:, jc * 512 : (jc + 1) * 512],
                    start=(it == 0),
                    stop=(it == NT - 1),
                )
            nc.vector.scalar_tensor_tensor(
                out=vdst[:, jc * 8 : (jc + 1) * 8, 0:DH],
                in0=mm.rearrange("p (g c) -> p g c", c=DH),
                scalar=0.0,
                in1=bvb_g[:, jc * 8 : (jc + 1) * 8, :],
                op0=mybir.AluOpType.add,
                op1=mybir.AluOpType.add,
            )

    # ---- projections for one jt (Q^T and K^T rows jt*128..) ----
    qk_tiles = {}  # jt -> (qT, kT)

    def emit_proj(jt, scs=(0, 1)):
        if jt not in qk_tiles:
            qk_tiles[jt] = tuple(
                qk_pool.tile([P, S], BF16, name=f"{nm}T", tag="qk")
                for nm in ("bq", "bk")
            )
        for dst, nm, wkey in zip(qk_tiles[jt], ("bq", "bk"), ("wq", "wk")):
            # per-half psum tiles (own 1-bank tag: proj never waits on the
            # scores ring, so ACT keeps draining exps during projections)
            for sc in scs:
                mm = ps_proj.tile([P, 512], F32, name="mmp", tag="proj")
                for it in range(NT):
                    nc.tensor.matmul(
                        mm,
                        lhsT=w_bf[(wkey, jt)][:, it * P : (it + 1) * P],
                        rhs=xTs(it)[:, sc * 512 : (sc + 1) * 512],
                        start=(it == 0),
                        stop=(it == NT - 1),
                    )
                nc.vector.tensor_scalar_add(
                    dst[:, sc * 512 : (sc + 1) * 512],
                    mm,
                    b_cols[nm][:, jt : jt + 1],
                )

    # ---- scores + exp for one head (S^T[k,q] by kt tile) ----
    pT_store = {}  # h -> list of 8 pT tiles

    def emit_ctx_qt(h, pT, bounce, qt):
        cps = ps_ctx.tile([P, VW], F32, name="cps", tag="ctx")
        for kt in range(NT):
            nc.tensor.matmul(
                cps,
                lhsT=pT[kt][:, qt * P : (qt + 1) * P],
                rhs=v_sb[kt][:, h * VW : (h + 1) * VW],
                start=(kt == 0),
                stop=(kt == NT - 1),
            )
        r = small_pool.tile([P, 1], F32, name="recip", tag="recip")
        nc.vector.reciprocal(r, cps[:, DH : DH + 1])
        nc.vector.tensor_scalar_mul(
            bounce[:, qt * DH : (qt + 1) * DH], cps[:, 0:DH], r
        )

    def emit_ctx_out_dma(h, bounce):
        # one DMA per head: [p, qt, j] -> out[(qt p), h*64+j]
        nc.sync.dma_start(
            out=o_d.ap()
            .rearrange("(q p) d -> p q d", p=P)[:, :, h * DH : (h + 1) * DH],
            in_=bounce.rearrange("p (q j) -> p q j", j=DH),
        )

    def emit_scores_exp(h, ctx_heads=()):
        # scores+exp for head h, with the qt-blocks of lagged ctx heads
        # interleaved between kt-blocks: PE chews ctx matmuls while ACT
        # drains the scores psum ring instead of head-of-line stalling.
        jt, ro = h // 2, (h % 2) * DH
        qT_t, kT_t = qk_tiles[jt]
        jobs = [(c, pT_store.pop(c),
                 bounce_pool.tile([P, NT * DH], F32, name="bounce", tag="bounce"))
                for c in ctx_heads]
        pT = []
        for kt in range(NT):
            sps = ps_sps.tile([P, S], F32, name="sps", tag="sps")
            lhsT = kT_t[ro : ro + DH, kt * P : (kt + 1) * P]
            for qc in range(SC):
                nc.tensor.matmul(
                    sps[:, qc * 512 : (qc + 1) * 512],
                    lhsT=lhsT,
                    rhs=qT_t[ro : ro + DH, qc * 512 : (qc + 1) * 512],
                    start=True,
                    stop=True,
                )
            pt = p_pool.tile([P, S], BF16, name="pT", tag="pT")
            nc.scalar.activation(
                pt,
                sps,
                mybir.ActivationFunctionType.Exp,
                bias=nonlocal_store["mask_cols"][:, kt : kt + 1],
                scale=SCALE,
            )
            pT.append(pt)
            for c, cpT, bounce in jobs:
                emit_ctx_qt(c, cpT, bounce, kt)
        pT_store[h] = pT
        for c, cpT, bounce in jobs:
            emit_ctx_out_dma(c, bounce)

    # ---- standalone ctx (tail) ----
    def emit_ctx(h):
        pT = pT_store.pop(h)
        bounce = bounce_pool.tile([P, NT * DH], F32, name="bounce", tag="bounce")
        for qt in range(NT):
            emit_ctx_qt(h, pT, bounce, qt)
        emit_ctx_out_dma(h, bounce)

    # ---- pipelined schedule ----
    # ctx[h] lags scores/exp[h] by CTX_LAG heads mid-stream (pT buffering),
    # catching down to lag 1 at the tail so only ctx[15] runs after exp[15].
    next_ctx = [0]

    def emit_head(h):
        lag = min(CTX_LAG, max(1, H - 1 - h))
        ctx_heads = []
        while next_ctx[0] <= h - lag:
            ctx_heads.append(next_ctx[0])
            next_ctx[0] += 1
        emit_scores_exp(h, ctx_heads)

    emit_xt(0, NT)
    cvt_wjt(0)
    emit_proj(0)
    cvt_wjt(1)
    emit_head(0)
    cvt_wjt(2)
    emit_head(1)
    cvt_wv()
    emit_bvb()
    emit_proj(1)
    # V interleaved with heads 2,3: PE chews V while ACT drains the exp
    # backlog of heads 0,1; all of V lands before ctx[0] (h=3).
    for st in range(0, 4):
        emit_v_tile(st)
    emit_head(2)
    for st in range(4, NT):
        emit_v_tile(st)
    emit_head(3)
    for jt in range(2, NT):
        emit_proj(jt)
        if jt + 1 < NT:
            cvt_wjt(jt + 1)
        emit_head(2 * jt)
        emit_head(2 * jt + 1)
    while next_ctx[0] < H:
        emit_ctx(next_ctx[0])
        next_ctx[0] += 1


def build_program(n_reps: int = 1, n_loop: int = 0) -> bass.Bass:
    nc = bacc.Bacc(trn_type="TRN2", target_bir_lowering=False, debug=False)

    x_d = nc.declare_dram_parameter("hidden_states", [S, D], F32, isOutput=False)
    m_d = nc.declare_dram_parameter("attention_mask", [S], F32, isOutput=False)
    wq_d = nc.declare_dram_parameter("Wq", [D, D], F32, isOutput=False)
    bq_d = nc.declare_dram_parameter("bq", [D], F32, isOutput=False)
    wk_d = nc.declare_dram_parameter("Wk", [D, D], F32, isOutput=False)
    bk_d = nc.declare_dram_parameter("bk", [D], F32, isOutput=False)
    wv_d = nc.declare_dram_parameter("Wv", [D, D], F32, isOutput=False)
    bv_d = nc.declare_dram_parameter("bv", [D], F32, isOutput=False)
    o_d = nc.declare_dram_parameter("out", [S, D], F32, isOutput=True)
    dram = (x_d, m_d, wq_d, bq_d, wk_d, bk_d, wv_d, bv_d, o_d)

    with tile.TileContext(nc) as tc:
        with (
            tc.tile_pool(name="consts", bufs=1) as cst,
            tc.tile_pool(name="xT", bufs=1) as xT_pool,
            tc.tile_pool(name="wpool", bufs=1) as w_pool,
            tc.tile_pool(name="qk", bufs=4) as qk_pool,
            tc.tile_pool(name="vsb", bufs=1) as v_pool,
            tc.tile_pool(name="xstage", bufs=2) as x_stage,
            # [128,1024]f32 rings: 1.5 jt's worth of Wq+Wk column-block
            # prefetch; Wv full rows get their own ring.
            tc.tile_pool(name="wstage", bufs=2) as w_stage,
            tc.tile_pool(name="wvstage", bufs=2) as wv_stage,
            tc.tile_pool(name="pT", bufs=8 * (CTX_LAG + 1)) as p_pool,
            tc.tile_pool(name="small", bufs=16) as small_pool,
            tc.tile_pool(name="bounce", bufs=2) as bounce_pool,
            # PSUM (8 banks): scores [128,1024] x2 = 4, proj [128,512] x2
            # = 2, ctx [128,65] x2 = 2.
            tc.tile_pool(name="pssps", bufs=2, space="PSUM") as ps_sps,
            tc.tile_pool(name="psproj", bufs=2, space="PSUM") as ps_proj,
            tc.tile_pool(name="psctx", bufs=2, space="PSUM") as ps_ctx,
        ):
            ident = cst.tile([P, P], F32, name="ident", tag="ident")
            make_identity(nc, ident)
            pools = (cst, xT_pool, w_pool, qk_pool, v_pool, x_stage, w_stage,
                     wv_stage, p_pool, small_pool, bounce_pool, ps_sps,
                     ps_proj, ps_ctx, ident)
            if n_loop:
                with tc.For_i(0, n_loop, 1):
                    emit_body(nc, dram, pools)
            else:
                for _ in range(n_reps):
                    emit_body(nc, dram, pools)
    nc.compile()
    return nc


_NC_CACHE = None


def _get_nc():
    global _NC_CACHE
    if _NC_CACHE is None:
        _NC_CACHE = build_program()
    return _NC_CACHE


def make_in_maps(hidden_states, attention_mask, Wq, bq, Wk, bk, Wv, bv):
    hs = np.ascontiguousarray(np.asarray(hidden_states, dtype=np.float32))
    am = np.ascontiguousarray(
        np.asarray(attention_mask, dtype=np.float32).reshape(B, S)
    )
    shared = {
        "Wq": np.ascontiguousarray(np.asarray(Wq, dtype=np.float32)),
        "bq": np.ascontiguousarray(np.asarray(bq, dtype=np.float32)),
        "Wk": np.ascontiguousarray(np.asarray(Wk, dtype=np.float32)),
        "bk": np.ascontiguousarray(np.asarray(bk, dtype=np.float32)),
        "Wv": np.ascontiguousarray(np.asarray(Wv, dtype=np.float32)),
        "bv": np.ascontiguousarray(np.asarray(bv, dtype=np.float32)),
    }
    return [
        {"hidden_states": hs[b], "attention_mask": am[b], **shared}
        for b in range(B)
    ]


def kernel(hidden_states, attention_mask, Wq, bq, Wk, bk, Wv, bv):
    nc = _get_nc()
    in_maps = make_in_maps(hidden_states, attention_mask, Wq, bq, Wk, bk, Wv, bv)
    res = run_bass_kernel_spmd(nc, in_maps, list(range(N_CORES))).results
    out = np.stack([np.asarray(res[b]["out"], dtype=np.float32) for b in range(B)])
    return out
